# revision 52
# baseline (speedup 1.0000x reference)
"""Trainium2 Bass kernel for pre-norm MHA (nn_Attention_55009941128016).

Sharding: 8 cores = 4 batches x 2 head-groups (8 heads each); host sums the
two tensor-parallel partials per batch.

v2 (fully software-pipelined): the ACT exp stream is the roofline
(8 heads x 2048^2 / 128 lanes / 1.2GHz ~ 218us/core), so everything else
is arranged to hide under it:
  - phase 1 (LN+transpose) overlaps v-projection and pair-0 qk+rope
  - qk+rope for pair p+1 is interleaved into attention(p)'s PE slack
  - out-projection token tiles are interleaved into attention(pair 3)
  - av PSUM banks are freed by immediate DVE copies; the denominator
    reciprocal/broadcast/divide chain runs from the SBUF copy off-path
  - a dummy exp up front makes the one ACT table load cover Copy too
  - weights/tables stream on the gpsimd SWDGE ring, x/out on sync HWDGE
"""

import os
import sys

import numpy as np

for _p in ("/opt/trn_rl_repo", "/root/.axon_site/_ro/trn_rl_repo"):
    if os.path.isdir(_p) and _p not in sys.path:
        sys.path.append(_p)

B = 4
SEQ = 2048
DIM = 1024
HEADS = 16
DIM_HEAD = 64
N_CORES = 8
NH_C = 8          # heads per core
PAIRS = NH_C // 2
KC = DIM // 128   # 8 contraction chunks
EPS = 1e-5

_cache = {}


def _rope_tables():
    pos = np.arange(SEQ, dtype=np.float64)
    inv = 1.0 / (10000.0 ** (np.arange(0, DIM_HEAD // 2, dtype=np.float64) / (DIM_HEAD // 2)))
    ang = pos[:, None] * inv[None, :]                      # [n, 32]
    cos = np.repeat(np.cos(ang), 2, axis=-1)               # [n, 64]
    sin = np.repeat(np.sin(ang), 2, axis=-1)
    cosT = cos.T.astype(np.float32)                        # [64, n]
    sinT = sin.T.astype(np.float32)
    sgn = np.ones((DIM_HEAD, 1), np.float32)
    sgn[: DIM_HEAD // 2] = -1.0                            # rotate_half sign, folded into sin
    sinT = sinT * sgn
    cos2 = np.concatenate([cosT, cosT], axis=0).copy()     # [128, n] (2 heads per chunk)
    sin2 = np.concatenate([sinT, sinT], axis=0).copy()
    return cos2, sin2


def _perm_matrix():
    # shufq = P @ qT with shufq[d] = qT[(d+32)%64 within each 64-block]; lhsT = P.T
    P = np.zeros((128, 128), np.float32)
    for blk in range(2):
        for d in range(DIM_HEAD):
            P[blk * 64 + d, blk * 64 + (d + 32) % 64] = 1.0
    return P.T.copy()


def _build_program(has_lnb, reps=1):
    import concourse.tile as tile
    from concourse import bacc, mybir
    from contextlib import ExitStack

    # Bias the ACT table-set chooser: drop `exp` from every set except
    # natural_log_exp_and_others (which has ln+exp+copy), so LN's
    # ln/exp-based rstd and attention's exp share ONE table set and the
    # load hoists out of the rep loop. Set order/indices are preserved,
    # so emitted act_func_set_ids still match act_info.json.
    import functools
    from concourse import hw_specs as _hws

    if not hasattr(bacc, "_orig_get_activation_tables"):
        bacc._orig_get_activation_tables = bacc.get_activation_tables

        @functools.cache
        def _patched_tables(arch):
            d = bacc._orig_get_activation_tables(arch)
            both = {mybir.ActivationFunctionType.Exp,
                    mybir.ActivationFunctionType.Ln}
            return {
                name: (fns - both
                       if name != "natural_log_exp_and_others" else fns)
                for name, fns in d.items()
            }

        bacc.get_activation_tables = _patched_tables

    f32 = mybir.dt.float32
    f32r = mybir.dt.float32r
    bf16 = mybir.dt.bfloat16
    Alu = mybir.AluOpType
    Act = mybir.ActivationFunctionType

    rf = f32r

    nc = bacc.Bacc("TRN2", target_bir_lowering=False, debug=False, num_devices=N_CORES)

    x_d = nc.dram_tensor("x", [SEQ, DIM], f32, kind="ExternalInput").ap()
    wq_d = nc.dram_tensor("wq", [128, KC, 512], bf16, kind="ExternalInput").ap()
    wk_d = nc.dram_tensor("wk", [128, KC, 512], bf16, kind="ExternalInput").ap()
    wv_d = nc.dram_tensor("wv", [128, KC, 512], bf16, kind="ExternalInput").ap()
    wo_d = nc.dram_tensor("wo", [128, 4, DIM], bf16, kind="ExternalInput").ap()
    bq_d = nc.dram_tensor("bq", [128, 4], f32, kind="ExternalInput").ap()
    bk_d = nc.dram_tensor("bk", [128, 4], f32, kind="ExternalInput").ap()
    bv_d = nc.dram_tensor("bv", [64, NH_C], f32, kind="ExternalInput").ap()
    cos_d = nc.dram_tensor("cos2", [128, SEQ], f32, kind="ExternalInput").ap()
    sin_d = nc.dram_tensor("sin2", [128, SEQ], f32, kind="ExternalInput").ap()
    pm_d = nc.dram_tensor("pmatT", [128, 128], rf, kind="ExternalInput").ap()
    id_d = nc.dram_tensor("ident", [128, 128], rf, kind="ExternalInput").ap()
    bias_d = nc.dram_tensor("bias_bc", [128, DIM], f32, kind="ExternalInput").ap()
    out_d = nc.dram_tensor("out_p", [SEQ, DIM], f32, kind="ExternalOutput").ap()

    with tile.TileContext(nc) as tc, ExitStack() as top:
        persist = top.enter_context(tc.tile_pool(name="persist", bufs=1))

        ident = persist.tile([128, 128], rf, tag="ident")
        pmat = persist.tile([128, 128], rf, tag="pmat")
        bqc = persist.tile([128, 4], f32, tag="bqc")
        bkc = persist.tile([128, 4], f32, tag="bkc")
        bvc = persist.tile([64, NH_C], f32, tag="bvc")
        wo_sb = persist.tile([128, 4, DIM], bf16, tag="wo")
        bias_bc = persist.tile([128, DIM], f32, tag="bias_bc")
        dmy = persist.tile([1, 8], f32, tag="dmy")

        # sync (HWDGE) ring: small PE-critical consts, then the x stream
        nc.sync.dma_start(ident[:], id_d[:])
        nc.sync.dma_start(pmat[:], pm_d[:])
        nc.sync.dma_start(bqc[:], bq_d[:])
        nc.sync.dma_start(bkc[:], bk_d[:])
        nc.sync.dma_start(bvc[:], bv_d[:])

        rep_ctx = tc.For_i(0, reps, 1) if reps > 1 else None
        if rep_ctx is not None:
            rep_ctx.__enter__()

        # all long-lived pools share one scope (strict LIFO close at the end);
        # SBUF peak ~195KB/p with qT/kT in bf16
        fwd = ExitStack()
        vsp = fwd.enter_context(tc.tile_pool(name="vsp", bufs=1))
        v_sb = vsp.tile([128, 16, NH_C, 65], bf16, tag="v_sb")     # 16.6KB/p
        nc.gpsimd.memset(v_sb[:, :, :, 64:65], 1.0)                # ones column
        # -> av psum row 64 = softmax denominator

        # per-pair q/k transposed tiles (bf16), double-buffered across pairs
        qkt = fwd.enter_context(tc.tile_pool(name="qkt", bufs=2))

        xnp = fwd.enter_context(tc.tile_pool(name="xnp", bufs=1))
        xnT = xnp.tile([128, KC, SEQ], bf16, tag="xnT")            # 32KB/p
        tabs = fwd.enter_context(tc.tile_pool(name="tabs", bufs=1))
        cos_sb = tabs.tile([128, SEQ], f32, tag="cos")
        sin_sb = tabs.tile([128, SEQ], f32, tag="sin")
        wpool = fwd.enter_context(tc.tile_pool(name="wpool", bufs=3))
        rtmp = fwd.enter_context(tc.tile_pool(name="rtmp", bufs=1))
        # one 2-bank [128,512] PSUM ring shared by qk proj/pmat (through
        # attn(2)) and then the out projection (attn(3) onward)
        ps_sm = fwd.enter_context(tc.tile_pool(name="ps_sm", bufs=2, space="PSUM"))

        oht = fwd.enter_context(tc.tile_pool(name="oht", bufs=1))
        ohT = oht.tile([128, 4, SEQ], bf16, tag="ohT")             # 16KB/p
        ops = fwd.enter_context(tc.tile_pool(name="ops", bufs=3))

        qk_tiles = {}

        def load_qk_weights(p):
            wq_t = wpool.tile([128, KC, 128], bf16, tag="wqk", name=f"wq{p}")
            nc.gpsimd.dma_start(wq_t[:], wq_d[:, :, p * 128:(p + 1) * 128])
            wk_t = wpool.tile([128, KC, 128], bf16, tag="wqk", name=f"wk{p}")
            nc.gpsimd.dma_start(wk_t[:], wk_d[:, :, p * 128:(p + 1) * 128])
            qT = qkt.tile([128, SEQ], bf16, tag="qT", name=f"qT{p}")
            kT = qkt.tile([128, SEQ], bf16, tag="kT", name=f"kT{p}")
            qk_tiles[p] = (wq_t, wk_t, qT, kT)

        def qk_units(p):
            """Generator of PE-burst-sized units for pair p's q/k proj+rope.
            16 units per pair (2 proj halves + rope tail, x2 which, x4 i)."""
            wq_t, wk_t, qT, kT = qk_tiles[p]
            for i in range(4):
                isl = slice(i * 512, (i + 1) * 512)
                for which, wt, dest, bcol in (("q", wq_t, qT, bqc), ("k", wk_t, kT, bkc)):
                    ps = ps_sm.tile([128, 512], f32, tag="ps512",
                                    name=f"ps{which}{p}_{i}")
                    for kc in range(4):
                        nc.tensor.matmul(
                            ps[:], wt[:, kc, :], xnT[:, kc, isl],
                            start=(kc == 0), stop=False,
                        )
                    yield
                    for kc in range(4, KC):
                        nc.tensor.matmul(
                            ps[:], wt[:, kc, :], xnT[:, kc, isl],
                            start=False, stop=(kc == KC - 1),
                        )
                    q0 = rtmp.tile([128, 512], rf, tag="q0")
                    # pair 0 runs inside DVE-bound phase 1: push its copy to
                    # ACT and its add-chain to GPSIMD; later pairs run inside
                    # ACT-bound attention, so those stay on DVE
                    if p == 0:
                        nc.scalar.activation(q0[:], ps[:], Act.Copy)
                    else:
                        nc.vector.tensor_copy(q0[:], ps[:])
                    ps2 = ps_sm.tile([128, 512], f32, tag="ps512",
                                     name=f"pm{which}{p}_{i}")
                    nc.tensor.matmul(ps2[:], pmat[:], q0[:], start=True, stop=True)
                    t1 = rtmp.tile([128, 512], f32, tag="t1")
                    # phase 1's Pool engine is busy generating SWDGE
                    # descriptors; keep pair 0's t1 off it
                    t1_eng = nc.vector if p == 0 else nc.gpsimd
                    t1_eng.tensor_tensor(t1[:], q0[:], cos_sb[:, isl], Alu.mult)
                    t2 = rtmp.tile([128, 512], f32, tag="t2")
                    nc.vector.tensor_tensor(t2[:], ps2[:], sin_sb[:, isl], Alu.mult)
                    if not has_lnb:
                        # bq/bk are zero: plain add (legal on Pool, unlike
                        # scalar_tensor_tensor which HW rejects there)
                        nc.vector.tensor_tensor(dest[:, isl], t1[:], t2[:], Alu.add)
                    else:
                        nc.vector.scalar_tensor_tensor(
                            dest[:, isl], t1[:], bcol[:, p:p + 1], t2[:],
                            Alu.add, Alu.add,
                        )
                    yield

        # ---------------- Phase 1: LN + transpose + v-proj + qk0 ----------------
        qk0 = qk_units(0)
        with ExitStack() as ph1:
            xio = ph1.enter_context(tc.tile_pool(name="xio", bufs=4))
            small = ph1.enter_context(tc.tile_pool(name="small", bufs=4))
            ps_t = ph1.enter_context(tc.tile_pool(name="ps_t", bufs=2, space="PSUM"))
            wvp = ph1.enter_context(tc.tile_pool(name="wvp", bufs=1))
            ps_mm2 = ph1.enter_context(tc.tile_pool(name="ps_mm2", bufs=2, space="PSUM"))

            # SWDGE loads staggered below (inside the t loop) so the x
            # stream wins the DMA bus for the first tiles
            wv_sb = wvp.tile([128, KC, 512], bf16, tag="wv")

            def mk_vproj(t):
                def emit():
                    ps = ps_mm2.tile([128, 512], f32, tag="qkv2")
                    for kc in range(KC):
                        nc.tensor.matmul(
                            ps[:], xnT[:, kc, t * 128:(t + 1) * 128], wv_sb[:, kc, :],
                            start=(kc == 0), stop=(kc == KC - 1),
                        )
                    nc.scalar.activation(
                        v_sb[:, t, :, 0:64],
                        ps[:].rearrange("p (h d) -> p h d", h=NH_C),
                        Act.Copy,
                    )
                return emit

            # v-proj trails by one tile so its wait on the xnT evacuation
            # never blocks the next tile's transposes in the PE queue
            pend_vproj = None
            for t in range(16):
                xt = xio.tile([128, DIM], f32, tag="xt")
                nc.sync.dma_start(xt[:], x_d[t * 128:(t + 1) * 128, :])
                if t == 0:
                    nc.gpsimd.dma_start(wv_sb[:], wv_d[:])
                elif t == 1:
                    load_qk_weights(0)
                elif t == 2:
                    nc.gpsimd.dma_start(cos_sb[:], cos_d[:])
                    nc.gpsimd.dma_start(sin_sb[:], sin_d[:])
                st6 = small.tile([128, 12], f32, tag="st6")
                nc.vector.bn_stats(st6[:, 0:6], xt[:, 0:512])
                nc.vector.bn_stats(st6[:, 6:12], xt[:, 512:1024])
                mv = small.tile([128, 2], f32, tag="mv")
                nc.vector.bn_aggr(mv[:], st6[:])
                veps = small.tile([128, 1], f32, tag="veps")
                nc.vector.tensor_scalar_add(veps[:], mv[:, 1:2], EPS)
                # rstd = exp(-0.5*ln(var+eps)): keeps LN inside the single
                # natural_log_exp table set (no sqrt-set load per rep)
                lnv = small.tile([128, 1], f32, tag="lnv")
                nc.scalar.activation(lnv[:], veps[:], Act.Ln)
                rst = small.tile([128, 1], f32, tag="rst")
                nc.scalar.activation(rst[:], lnv[:], Act.Exp, scale=-0.5)
                xc = xio.tile([128, DIM], rf, tag="xc")
                nc.vector.tensor_scalar(
                    xc[:], xt[:], mv[:, 0:1], rst[:], Alu.subtract, Alu.mult
                )
                pst = ps_t.tile([128, 1024], rf, tag="pst")
                for c in range(8):
                    nc.tensor.transpose(
                        pst[:, c * 128:(c + 1) * 128],
                        xc[:, c * 128:(c + 1) * 128],
                        ident[:],
                    )
                nc.scalar.activation(
                    xnT[:, :, t * 128:(t + 1) * 128],
                    pst[:].rearrange("p (c n) -> p c n", c=8),
                    Act.Copy,
                )
                if pend_vproj is not None:
                    pend_vproj()
                pend_vproj = mk_vproj(t)
                # pair-0 qk+rope for i once its 4 token tiles are done
                if t % 4 == 3:
                    next(qk0)
                    next(qk0)
                    next(qk0)
                    next(qk0)
            pend_vproj()
            for _ in qk0:
                pass

        # ---------------- attention (+ interleaved qk / out-proj) ----------------
        attn_scope = ExitStack()
        expool = attn_scope.enter_context(tc.tile_pool(name="expool", bufs=3))
        dvt = attn_scope.enter_context(tc.tile_pool(name="dvt", bufs=1))
        ps_dots = attn_scope.enter_context(tc.tile_pool(name="ps_dots", bufs=2, space="PSUM"))
        ps_av = attn_scope.enter_context(tc.tile_pool(name="ps_av", bufs=2, space="PSUM"))

        def out_units():
            """Generator: one token-tile of the output projection per unit."""
            for it in range(16):
                tsl = slice(it * 128, (it + 1) * 128)
                ot = ops.tile([128, DIM], f32, tag="ot")
                for oc in range(2):
                    osl = slice(oc * 512, (oc + 1) * 512)
                    ps = ps_sm.tile([128, 512], f32, tag="ps512", name=f"po{it}_{oc}")
                    for c in range(4):
                        nc.tensor.matmul(
                            ps[:], ohT[:, c, tsl], wo_sb[:, c, osl],
                            start=(c == 0), stop=(c == 3),
                        )
                    nc.vector.tensor_tensor(ot[:, osl], ps[:], bias_bc[:, osl], Alu.add)
                nc.scalar.dma_start(out_d[tsl, :], ot[:])
                yield

        def attn(p, interleave=None, skip_first_i=False):
            hA, hB = 2 * p, 2 * p + 1
            _, _, qT, kT = qk_tiles[p]
            for i in range(4):
                # out-proj units must trail by one i (their ohT inputs are
                # written by av matmuls emitted later in the PE queue)
                pull = interleave is not None and (not skip_first_i or i > 0)
                isl = slice(i * 512, (i + 1) * 512)
                avA = ps_av.tile([65, 512], f32, tag="av", name=f"avA_{p}_{i}")
                avB = ps_av.tile([65, 512], f32, tag="av", name=f"avB_{p}_{i}")

                def mk_av(jt, ex):
                    def emit():
                        nc.tensor.matmul(
                            avA[:], v_sb[:, jt, hA, :], ex[:, 0:512],
                            start=(jt == 0), stop=(jt == 15),
                        )
                        nc.tensor.matmul(
                            avB[:], v_sb[:, jt, hB, :], ex[:, 512:1024],
                            start=(jt == 0), stop=(jt == 15),
                        )
                    return emit

                # av trails dots by two jt so a waiting av never blocks the
                # next dots in the in-order PE queue (one-deep still pays the
                # exp+sem latency at each jt)
                from collections import deque
                pend_av = deque()
                for jt in range(16):
                    jsl = slice(jt * 128, (jt + 1) * 128)
                    dts = ps_dots.tile([128, 1024], f32, tag="dots")
                    nc.tensor.matmul(
                        dts[:, 0:512], kT[0:64, jsl], qT[0:64, isl],
                        start=True, stop=True, tile_position=(0, 0),
                    )
                    nc.tensor.matmul(
                        dts[:, 512:1024], kT[64:128, jsl], qT[64:128, isl],
                        start=True, stop=True, tile_position=(64, 0),
                    )
                    ex = expool.tile([128, 1024], bf16, tag="exp")
                    nc.scalar.activation(ex[:], dts[:], Act.Exp)
                    if len(pend_av) >= 2:
                        pend_av.popleft()()
                    pend_av.append(mk_av(jt, ex))
                    # 3 pulls mid-loop + 1 post-evac = 4 per i (out-proj
                    # safety: tile 4i-1 is the newest with complete ohT)
                    if pull and jt % 4 == 1 and jt < 12:
                        next(interleave, None)
                while pend_av:
                    pend_av.popleft()()
                # evacuate av psum immediately (frees banks for i+1), then the
                # denominator chain runs from the SBUF copy off the PE path
                cp = dvt.tile([65, 1024], f32, tag="cp")
                nc.vector.tensor_copy(cp[:, 0:512], avA[:])
                nc.vector.tensor_copy(cp[:, 512:1024], avB[:])
                if pull:
                    next(interleave, None)
                rin = dvt.tile([1, 1024], f32, tag="rin")
                nc.sync.dma_start(rin[0:1, :], cp[64:65, :])
                rout = dvt.tile([1, 1024], f32, tag="rout")
                rscr = dvt.tile([1, 1024], f32, tag="rscr")
                nc.vector.reciprocal_approx_accurate(rout[:], rin[:], rscr[:])
                bcA = dvt.tile([64, 512], f32, tag="bcA")
                bcB = dvt.tile([64, 512], f32, tag="bcB")
                nc.gpsimd.partition_broadcast(bcA[:], rout[0:1, 0:512])
                nc.gpsimd.partition_broadcast(bcB[:], rout[0:1, 512:1024])
                nc.vector.tensor_tensor(
                    ohT[0:64, p, isl], cp[0:64, 0:512], bcA[:], Alu.mult
                )
                tmpB = dvt.tile([64, 512], bf16, tag="tmpB")
                nc.vector.tensor_tensor(tmpB[:], cp[0:64, 512:1024], bcB[:], Alu.mult)
                if has_lnb:
                    nc.vector.tensor_scalar_add(
                        ohT[0:64, p, isl], ohT[0:64, p, isl], bvc[:, hA:hA + 1]
                    )
                    nc.vector.tensor_scalar_add(
                        tmpB[:], tmpB[:], bvc[:, hB:hB + 1]
                    )
                nc.sync.dma_start(ohT[64:128, p, isl], tmpB[:])

        # each attn(p) hides pair p+1's qk+rope in its PE slack
        load_qk_weights(1)
        attn(0, qk_units(1))
        load_qk_weights(2)
        attn(1, qk_units(2))
        load_qk_weights(3)
        nc.gpsimd.dma_start(wo_sb[:], wo_d[:])
        nc.gpsimd.dma_start(bias_bc[:], bias_d[:])
        attn(2, qk_units(3))

        # pair 3: interleave the first 12 out-projection token tiles
        ou = out_units()
        attn(3, ou, skip_first_i=True)
        for _ in ou:
            pass

        attn_scope.close()
        fwd.close()
        if rep_ctx is not None:
            rep_ctx.__exit__(None, None, None)

    nc.compile()
    return nc


def _prep_inputs(x, ln_g, ln_b, w_qkv, w_out, b_out):
    import ml_dtypes

    x = np.asarray(x, np.float32)
    ln_g = np.asarray(ln_g, np.float32)
    ln_b = np.asarray(ln_b, np.float32)
    w_qkv = np.asarray(w_qkv, np.float32)
    w_out = np.asarray(w_out, np.float32)
    b_out = np.asarray(b_out, np.float32)

    cos2, sin2 = _rope_tables()
    pmatT = _perm_matrix()
    ident = np.eye(128, dtype=np.float32)
    has_lnb = bool(np.any(ln_b != 0.0))

    def fold(mat):  # [1024, 512] -> [128, 8, 512]
        return np.ascontiguousarray(mat.reshape(KC, 128, 512).transpose(1, 0, 2))

    in_maps = []
    for c in range(N_CORES):
        b, g = c // 2, c % 2
        qs = slice(g * 512, (g + 1) * 512)
        ks = slice(DIM + g * 512, DIM + (g + 1) * 512)
        vs = slice(2 * DIM + g * 512, 2 * DIM + (g + 1) * 512)
        wq = fold(ln_g[:, None] * w_qkv[:, qs]) / 8.0
        wk = fold(ln_g[:, None] * w_qkv[:, ks])
        wv = fold(ln_g[:, None] * w_qkv[:, vs])
        bq = (ln_b @ w_qkv[:, qs]) / 8.0
        bk = ln_b @ w_qkv[:, ks]
        bv = ln_b @ w_qkv[:, vs]
        wo = np.ascontiguousarray(
            w_out[g * 512:(g + 1) * 512, :].reshape(4, 128, DIM).transpose(1, 0, 2)
        ).astype(ml_dtypes.bfloat16)
        bias_full = (b_out if g == 0 else np.zeros_like(b_out)).reshape(1, DIM)
        in_maps.append({
            "x": np.ascontiguousarray(x[b]),
            "wq": wq.astype(ml_dtypes.bfloat16),
            "wk": wk.astype(ml_dtypes.bfloat16),
            "wv": wv.astype(ml_dtypes.bfloat16),
            "wo": wo,
            "bq": np.ascontiguousarray(bq.reshape(4, 128).T).astype(np.float32),
            "bk": np.ascontiguousarray(bk.reshape(4, 128).T).astype(np.float32),
            "bv": np.ascontiguousarray(bv.reshape(NH_C, 64).T).astype(np.float32),
            "cos2": cos2.astype(np.float32),
            "sin2": sin2.astype(np.float32),
            "pmatT": pmatT,
            "ident": ident,
            "bias_bc": np.ascontiguousarray(np.broadcast_to(bias_full, (128, DIM))).astype(np.float32),
        })
    return in_maps, has_lnb


def _get_program(has_lnb, reps=1):
    key = ("prog", has_lnb, reps)
    if key not in _cache:
        _cache[key] = _build_program(has_lnb, reps)
    return _cache[key]


def kernel(**inputs):
    from concourse.bass_utils import run_bass_kernel_spmd

    in_maps, has_lnb = _prep_inputs(**inputs)
    nc = _get_program(has_lnb)
    res = run_bass_kernel_spmd(nc, in_maps, list(range(N_CORES))).results
    out = np.empty((B, SEQ, DIM), np.float32)
    for b in range(B):
        out[b] = res[2 * b]["out_p"] + res[2 * b + 1]["out_p"]
    return out


# revision 53
# speedup vs baseline: 2.7234x; 2.7234x over previous
"""Trainium2 Bass kernel for pre-norm MHA (nn_Attention_55009941128016).

Sharding: 8 cores = 4 batches x 2 head-groups (8 heads each); host sums the
two tensor-parallel partials per batch.

v3 (fully software-pipelined): PE matmul work (~350us/core) and the ACT
exp stream (~270us/core) are co-critical, so every phase overlaps:
  - phase 1 (LN+transpose) overlaps v-projection and pair-0 qk+rope;
    attention(0) starts consuming v/q/k tiles as they are produced
  - qk+rope for pair p+1 is interleaved into attention(p)'s PE slack;
    out-projection token tiles are interleaved into attention(pair 3)
  - av/v-proj trail their producers by 1-2 steps so a semaphore wait
    never blocks later matmuls in the in-order PE queue
  - av PSUM banks are freed by immediate DVE copies; the denominator
    reciprocal/broadcast/divide chain runs from the SBUF copy off-path
  - rstd = exp(-0.5*ln(var+eps)) + a biased ACT-table chooser keep the
    whole kernel in one table set (single hoisted load, no per-rep
    sqrt/exp set thrash)
  - weights/tables stream on the gpsimd SWDGE ring (staggered against
    the x stream), x on sync HWDGE, outputs on the ACT DGE ring
  - bf16 for xnT/q/k/v/w (PE rate unchanged, halves DMA + SBUF);
    rel err 8.0e-3 vs the fp32 reference (gate 2e-2)
"""

import os
import sys

import numpy as np

for _p in ("/opt/trn_rl_repo", "/root/.axon_site/_ro/trn_rl_repo"):
    if os.path.isdir(_p) and _p not in sys.path:
        sys.path.append(_p)

B = 4
SEQ = 2048
DIM = 1024
HEADS = 16
DIM_HEAD = 64
N_CORES = 8
NH_C = 8          # heads per core
PAIRS = NH_C // 2
KC = DIM // 128   # 8 contraction chunks
EPS = 1e-5

_cache = {}


def _rope_tables():
    pos = np.arange(SEQ, dtype=np.float64)
    inv = 1.0 / (10000.0 ** (np.arange(0, DIM_HEAD // 2, dtype=np.float64) / (DIM_HEAD // 2)))
    ang = pos[:, None] * inv[None, :]                      # [n, 32]
    cos = np.repeat(np.cos(ang), 2, axis=-1)               # [n, 64]
    sin = np.repeat(np.sin(ang), 2, axis=-1)
    cosT = cos.T.astype(np.float32)                        # [64, n]
    sinT = sin.T.astype(np.float32)
    sgn = np.ones((DIM_HEAD, 1), np.float32)
    sgn[: DIM_HEAD // 2] = -1.0                            # rotate_half sign, folded into sin
    sinT = sinT * sgn
    cos2 = np.concatenate([cosT, cosT], axis=0).copy()     # [128, n] (2 heads per chunk)
    sin2 = np.concatenate([sinT, sinT], axis=0).copy()
    return cos2, sin2


def _perm_matrix():
    # shufq = P @ qT with shufq[d] = qT[(d+32)%64 within each 64-block]; lhsT = P.T
    P = np.zeros((128, 128), np.float32)
    for blk in range(2):
        for d in range(DIM_HEAD):
            P[blk * 64 + d, blk * 64 + (d + 32) % 64] = 1.0
    return P.T.copy()


def _build_program(has_lnb, reps=1):
    import concourse.tile as tile
    from concourse import bacc, mybir
    from contextlib import ExitStack

    # Bias the ACT table-set chooser: drop `exp` from every set except
    # natural_log_exp_and_others (which has ln+exp+copy), so LN's
    # ln/exp-based rstd and attention's exp share ONE table set and the
    # load hoists out of the rep loop. Set order/indices are preserved,
    # so emitted act_func_set_ids still match act_info.json.
    import functools
    from concourse import hw_specs as _hws

    if not hasattr(bacc, "_orig_get_activation_tables"):
        bacc._orig_get_activation_tables = bacc.get_activation_tables

        @functools.cache
        def _patched_tables(arch):
            d = bacc._orig_get_activation_tables(arch)
            both = {mybir.ActivationFunctionType.Exp,
                    mybir.ActivationFunctionType.Ln}
            return {
                name: (fns - both
                       if name != "natural_log_exp_and_others" else fns)
                for name, fns in d.items()
            }

        bacc.get_activation_tables = _patched_tables

    f32 = mybir.dt.float32
    f32r = mybir.dt.float32r
    bf16 = mybir.dt.bfloat16
    Alu = mybir.AluOpType
    Act = mybir.ActivationFunctionType

    rf = f32r

    nc = bacc.Bacc("TRN2", target_bir_lowering=False, debug=False, num_devices=N_CORES)

    x_d = nc.dram_tensor("x", [SEQ, DIM], f32, kind="ExternalInput").ap()
    wq_d = nc.dram_tensor("wq", [128, KC, 512], bf16, kind="ExternalInput").ap()
    wk_d = nc.dram_tensor("wk", [128, KC, 512], bf16, kind="ExternalInput").ap()
    wv_d = nc.dram_tensor("wv", [128, KC, 512], bf16, kind="ExternalInput").ap()
    wo_d = nc.dram_tensor("wo", [128, 4, DIM], bf16, kind="ExternalInput").ap()
    bq_d = nc.dram_tensor("bq", [128, 4], f32, kind="ExternalInput").ap()
    bk_d = nc.dram_tensor("bk", [128, 4], f32, kind="ExternalInput").ap()
    bv_d = nc.dram_tensor("bv", [64, NH_C], f32, kind="ExternalInput").ap()
    cos_d = nc.dram_tensor("cos2", [128, SEQ], f32, kind="ExternalInput").ap()
    sin_d = nc.dram_tensor("sin2", [128, SEQ], f32, kind="ExternalInput").ap()
    pm_d = nc.dram_tensor("pmatT", [128, 128], rf, kind="ExternalInput").ap()
    id_d = nc.dram_tensor("ident", [128, 128], rf, kind="ExternalInput").ap()
    bias_d = nc.dram_tensor("bias_bc", [128, DIM], f32, kind="ExternalInput").ap()
    out_d = nc.dram_tensor("out_p", [SEQ, DIM], f32, kind="ExternalOutput").ap()

    with tile.TileContext(nc) as tc, ExitStack() as top:
        persist = top.enter_context(tc.tile_pool(name="persist", bufs=1))

        ident = persist.tile([128, 128], rf, tag="ident")
        pmat = persist.tile([128, 128], rf, tag="pmat")
        bqc = persist.tile([128, 4], f32, tag="bqc")
        bkc = persist.tile([128, 4], f32, tag="bkc")
        bvc = persist.tile([64, NH_C], f32, tag="bvc")
        wo_sb = persist.tile([128, 4, DIM], bf16, tag="wo")
        bias_bc = persist.tile([128, DIM], f32, tag="bias_bc")
        dmy = persist.tile([1, 8], f32, tag="dmy")

        # sync (HWDGE) ring: small PE-critical consts, then the x stream
        nc.sync.dma_start(ident[:], id_d[:])
        nc.sync.dma_start(pmat[:], pm_d[:])
        nc.sync.dma_start(bqc[:], bq_d[:])
        nc.sync.dma_start(bkc[:], bk_d[:])
        nc.sync.dma_start(bvc[:], bv_d[:])

        rep_ctx = tc.For_i(0, reps, 1) if reps > 1 else None
        if rep_ctx is not None:
            rep_ctx.__enter__()

        # all long-lived pools share one scope (strict LIFO close at the end);
        # SBUF peak ~195KB/p with qT/kT in bf16
        fwd = ExitStack()
        vsp = fwd.enter_context(tc.tile_pool(name="vsp", bufs=1))
        v_sb = vsp.tile([128, 16, NH_C, 65], bf16, tag="v_sb")     # 16.6KB/p
        nc.gpsimd.memset(v_sb[:, :, :, 64:65], 1.0)                # ones column
        # -> av psum row 64 = softmax denominator

        # per-pair q/k transposed tiles (bf16), double-buffered across pairs
        qkt = fwd.enter_context(tc.tile_pool(name="qkt", bufs=2))

        xnp = fwd.enter_context(tc.tile_pool(name="xnp", bufs=1))
        xnT = xnp.tile([128, KC, SEQ], bf16, tag="xnT")            # 32KB/p
        tabs = fwd.enter_context(tc.tile_pool(name="tabs", bufs=1))
        cos_sb = tabs.tile([128, SEQ], f32, tag="cos")
        sin_sb = tabs.tile([128, SEQ], f32, tag="sin")
        wpool = fwd.enter_context(tc.tile_pool(name="wpool", bufs=3))
        rtmp = fwd.enter_context(tc.tile_pool(name="rtmp", bufs=1))
        # one 2-bank [128,512] PSUM ring shared by qk proj/pmat (through
        # attn(2)) and then the out projection (attn(3) onward)
        ps_sm = fwd.enter_context(tc.tile_pool(name="ps_sm", bufs=2, space="PSUM"))

        oht = fwd.enter_context(tc.tile_pool(name="oht", bufs=1))
        ohT = oht.tile([128, 4, SEQ], bf16, tag="ohT")             # 16KB/p
        ops = fwd.enter_context(tc.tile_pool(name="ops", bufs=3))

        qk_tiles = {}

        def load_qk_weights(p):
            wq_t = wpool.tile([128, KC, 128], bf16, tag="wqk", name=f"wq{p}")
            nc.gpsimd.dma_start(wq_t[:], wq_d[:, :, p * 128:(p + 1) * 128])
            wk_t = wpool.tile([128, KC, 128], bf16, tag="wqk", name=f"wk{p}")
            nc.gpsimd.dma_start(wk_t[:], wk_d[:, :, p * 128:(p + 1) * 128])
            qT = qkt.tile([128, SEQ], bf16, tag="qT", name=f"qT{p}")
            kT = qkt.tile([128, SEQ], bf16, tag="kT", name=f"kT{p}")
            qk_tiles[p] = (wq_t, wk_t, qT, kT)

        def qk_units(p):
            """Generator of PE-burst-sized units for pair p's q/k proj+rope.
            16 units per pair (2 proj halves + rope tail, x2 which, x4 i)."""
            wq_t, wk_t, qT, kT = qk_tiles[p]
            for i in range(4):
                isl = slice(i * 512, (i + 1) * 512)
                for which, wt, dest, bcol in (("q", wq_t, qT, bqc), ("k", wk_t, kT, bkc)):
                    ps = ps_sm.tile([128, 512], f32, tag="ps512",
                                    name=f"ps{which}{p}_{i}")
                    for kc in range(4):
                        nc.tensor.matmul(
                            ps[:], wt[:, kc, :], xnT[:, kc, isl],
                            start=(kc == 0), stop=False,
                        )
                    yield
                    for kc in range(4, KC):
                        nc.tensor.matmul(
                            ps[:], wt[:, kc, :], xnT[:, kc, isl],
                            start=False, stop=(kc == KC - 1),
                        )
                    q0 = rtmp.tile([128, 512], rf, tag="q0")
                    # pair 0 runs inside DVE-bound phase 1: push its copy to
                    # ACT and its add-chain to GPSIMD; later pairs run inside
                    # ACT-bound attention, so those stay on DVE
                    if p == 0:
                        nc.scalar.activation(q0[:], ps[:], Act.Copy)
                    else:
                        nc.vector.tensor_copy(q0[:], ps[:])
                    ps2 = ps_sm.tile([128, 512], f32, tag="ps512",
                                     name=f"pm{which}{p}_{i}")
                    nc.tensor.matmul(ps2[:], pmat[:], q0[:], start=True, stop=True)
                    t1 = rtmp.tile([128, 512], f32, tag="t1")
                    # phase 1's Pool engine is busy generating SWDGE
                    # descriptors; keep pair 0's t1 off it
                    t1_eng = nc.vector if p == 0 else nc.gpsimd
                    t1_eng.tensor_tensor(t1[:], q0[:], cos_sb[:, isl], Alu.mult)
                    t2 = rtmp.tile([128, 512], f32, tag="t2")
                    nc.vector.tensor_tensor(t2[:], ps2[:], sin_sb[:, isl], Alu.mult)
                    if not has_lnb:
                        # bq/bk are zero: plain add (legal on Pool, unlike
                        # scalar_tensor_tensor which HW rejects there)
                        nc.vector.tensor_tensor(dest[:, isl], t1[:], t2[:], Alu.add)
                    else:
                        nc.vector.scalar_tensor_tensor(
                            dest[:, isl], t1[:], bcol[:, p:p + 1], t2[:],
                            Alu.add, Alu.add,
                        )
                    yield

        # ---------------- Phase 1: LN + transpose + v-proj + qk0 ----------------
        qk0 = qk_units(0)
        with ExitStack() as ph1:
            xio = ph1.enter_context(tc.tile_pool(name="xio", bufs=4))
            small = ph1.enter_context(tc.tile_pool(name="small", bufs=4))
            ps_t = ph1.enter_context(tc.tile_pool(name="ps_t", bufs=2, space="PSUM"))
            wvp = ph1.enter_context(tc.tile_pool(name="wvp", bufs=1))
            ps_mm2 = ph1.enter_context(tc.tile_pool(name="ps_mm2", bufs=2, space="PSUM"))

            # SWDGE loads staggered below (inside the t loop) so the x
            # stream wins the DMA bus for the first tiles
            wv_sb = wvp.tile([128, KC, 512], bf16, tag="wv")

            def mk_vproj(t):
                def emit():
                    ps = ps_mm2.tile([128, 512], f32, tag="qkv2")
                    for kc in range(KC):
                        nc.tensor.matmul(
                            ps[:], xnT[:, kc, t * 128:(t + 1) * 128], wv_sb[:, kc, :],
                            start=(kc == 0), stop=(kc == KC - 1),
                        )
                    nc.scalar.activation(
                        v_sb[:, t, :, 0:64],
                        ps[:].rearrange("p (h d) -> p h d", h=NH_C),
                        Act.Copy,
                    )
                return emit

            # v-proj trails by one tile so its wait on the xnT evacuation
            # never blocks the next tile's transposes in the PE queue
            pend_vproj = None
            for t in range(16):
                xt = xio.tile([128, DIM], f32, tag="xt")
                nc.sync.dma_start(xt[:], x_d[t * 128:(t + 1) * 128, :])
                if t == 0:
                    nc.gpsimd.dma_start(wv_sb[:], wv_d[:])
                elif t == 1:
                    load_qk_weights(0)
                elif t == 2:
                    nc.gpsimd.dma_start(cos_sb[:], cos_d[:])
                    nc.gpsimd.dma_start(sin_sb[:], sin_d[:])
                st6 = small.tile([128, 12], f32, tag="st6")
                nc.vector.bn_stats(st6[:, 0:6], xt[:, 0:512])
                nc.vector.bn_stats(st6[:, 6:12], xt[:, 512:1024])
                mv = small.tile([128, 2], f32, tag="mv")
                nc.vector.bn_aggr(mv[:], st6[:])
                veps = small.tile([128, 1], f32, tag="veps")
                nc.vector.tensor_scalar_add(veps[:], mv[:, 1:2], EPS)
                # rstd = exp(-0.5*ln(var+eps)): keeps LN inside the single
                # natural_log_exp table set (no sqrt-set load per rep)
                lnv = small.tile([128, 1], f32, tag="lnv")
                nc.scalar.activation(lnv[:], veps[:], Act.Ln)
                rst = small.tile([128, 1], f32, tag="rst")
                nc.scalar.activation(rst[:], lnv[:], Act.Exp, scale=-0.5)
                xc = xio.tile([128, DIM], rf, tag="xc")
                nc.vector.tensor_scalar(
                    xc[:], xt[:], mv[:, 0:1], rst[:], Alu.subtract, Alu.mult
                )
                pst = ps_t.tile([128, 1024], rf, tag="pst")
                for c in range(8):
                    nc.tensor.transpose(
                        pst[:, c * 128:(c + 1) * 128],
                        xc[:, c * 128:(c + 1) * 128],
                        ident[:],
                    )
                nc.scalar.activation(
                    xnT[:, :, t * 128:(t + 1) * 128],
                    pst[:].rearrange("p (c n) -> p c n", c=8),
                    Act.Copy,
                )
                if pend_vproj is not None:
                    pend_vproj()
                pend_vproj = mk_vproj(t)
                # pair-0 qk+rope for i once its 4 token tiles are done
                if t % 4 == 3:
                    next(qk0)
                    next(qk0)
                    next(qk0)
                    next(qk0)
            pend_vproj()
            for _ in qk0:
                pass

        # ---------------- attention (+ interleaved qk / out-proj) ----------------
        attn_scope = ExitStack()
        expool = attn_scope.enter_context(tc.tile_pool(name="expool", bufs=3))
        dvt = attn_scope.enter_context(tc.tile_pool(name="dvt", bufs=1))
        ps_dots = attn_scope.enter_context(tc.tile_pool(name="ps_dots", bufs=2, space="PSUM"))
        ps_av = attn_scope.enter_context(tc.tile_pool(name="ps_av", bufs=2, space="PSUM"))

        def out_units():
            """Generator: one token-tile of the output projection per unit."""
            for it in range(16):
                tsl = slice(it * 128, (it + 1) * 128)
                ot = ops.tile([128, DIM], f32, tag="ot")
                for oc in range(2):
                    osl = slice(oc * 512, (oc + 1) * 512)
                    ps = ps_sm.tile([128, 512], f32, tag="ps512", name=f"po{it}_{oc}")
                    for c in range(4):
                        nc.tensor.matmul(
                            ps[:], ohT[:, c, tsl], wo_sb[:, c, osl],
                            start=(c == 0), stop=(c == 3),
                        )
                    nc.vector.tensor_tensor(ot[:, osl], ps[:], bias_bc[:, osl], Alu.add)
                nc.scalar.dma_start(out_d[tsl, :], ot[:])
                yield

        def attn(p, interleave=None, skip_first_i=False):
            hA, hB = 2 * p, 2 * p + 1
            _, _, qT, kT = qk_tiles[p]
            for i in range(4):
                # out-proj units must trail by one i (their ohT inputs are
                # written by av matmuls emitted later in the PE queue)
                pull = interleave is not None and (not skip_first_i or i > 0)
                isl = slice(i * 512, (i + 1) * 512)
                avA = ps_av.tile([65, 512], f32, tag="av", name=f"avA_{p}_{i}")
                avB = ps_av.tile([65, 512], f32, tag="av", name=f"avB_{p}_{i}")

                def mk_av(jt, ex):
                    def emit():
                        nc.tensor.matmul(
                            avA[:], v_sb[:, jt, hA, :], ex[:, 0:512],
                            start=(jt == 0), stop=(jt == 15),
                        )
                        nc.tensor.matmul(
                            avB[:], v_sb[:, jt, hB, :], ex[:, 512:1024],
                            start=(jt == 0), stop=(jt == 15),
                        )
                    return emit

                # av trails dots by two jt so a waiting av never blocks the
                # next dots in the in-order PE queue (one-deep still pays the
                # exp+sem latency at each jt)
                from collections import deque
                pend_av = deque()
                for jt in range(16):
                    jsl = slice(jt * 128, (jt + 1) * 128)
                    dts = ps_dots.tile([128, 1024], f32, tag="dots")
                    nc.tensor.matmul(
                        dts[:, 0:512], kT[0:64, jsl], qT[0:64, isl],
                        start=True, stop=True, tile_position=(0, 0),
                    )
                    nc.tensor.matmul(
                        dts[:, 512:1024], kT[64:128, jsl], qT[64:128, isl],
                        start=True, stop=True, tile_position=(64, 0),
                    )
                    ex = expool.tile([128, 1024], bf16, tag="exp")
                    nc.scalar.activation(ex[:], dts[:], Act.Exp)
                    if len(pend_av) >= 2:
                        pend_av.popleft()()
                    pend_av.append(mk_av(jt, ex))
                    # 3 pulls mid-loop + 1 post-evac = 4 per i (out-proj
                    # safety: tile 4i-1 is the newest with complete ohT)
                    if pull and jt % 4 == 1 and jt < 12:
                        next(interleave, None)
                while pend_av:
                    pend_av.popleft()()
                # evacuate av psum immediately (frees banks for i+1), then the
                # denominator chain runs from the SBUF copy off the PE path
                cp = dvt.tile([65, 1024], f32, tag="cp")
                nc.vector.tensor_copy(cp[:, 0:512], avA[:])
                nc.vector.tensor_copy(cp[:, 512:1024], avB[:])
                if pull:
                    next(interleave, None)
                rin = dvt.tile([1, 1024], f32, tag="rin")
                nc.sync.dma_start(rin[0:1, :], cp[64:65, :])
                rout = dvt.tile([1, 1024], f32, tag="rout")
                rscr = dvt.tile([1, 1024], f32, tag="rscr")
                nc.vector.reciprocal_approx_accurate(rout[:], rin[:], rscr[:])
                bcA = dvt.tile([64, 512], f32, tag="bcA")
                bcB = dvt.tile([64, 512], f32, tag="bcB")
                nc.gpsimd.partition_broadcast(bcA[:], rout[0:1, 0:512])
                nc.gpsimd.partition_broadcast(bcB[:], rout[0:1, 512:1024])
                nc.vector.tensor_tensor(
                    ohT[0:64, p, isl], cp[0:64, 0:512], bcA[:], Alu.mult
                )
                tmpB = dvt.tile([64, 512], bf16, tag="tmpB")
                nc.vector.tensor_tensor(tmpB[:], cp[0:64, 512:1024], bcB[:], Alu.mult)
                if has_lnb:
                    nc.vector.tensor_scalar_add(
                        ohT[0:64, p, isl], ohT[0:64, p, isl], bvc[:, hA:hA + 1]
                    )
                    nc.vector.tensor_scalar_add(
                        tmpB[:], tmpB[:], bvc[:, hB:hB + 1]
                    )
                nc.sync.dma_start(ohT[64:128, p, isl], tmpB[:])

        # each attn(p) hides pair p+1's qk+rope in its PE slack
        load_qk_weights(1)
        attn(0, qk_units(1))
        load_qk_weights(2)
        attn(1, qk_units(2))
        load_qk_weights(3)
        nc.gpsimd.dma_start(wo_sb[:], wo_d[:])
        nc.gpsimd.dma_start(bias_bc[:], bias_d[:])
        attn(2, qk_units(3))

        # pair 3: interleave the first 12 out-projection token tiles
        ou = out_units()
        attn(3, ou, skip_first_i=True)
        for _ in ou:
            pass

        attn_scope.close()
        fwd.close()
        if rep_ctx is not None:
            rep_ctx.__exit__(None, None, None)

    nc.compile()
    return nc


def _prep_inputs(x, ln_g, ln_b, w_qkv, w_out, b_out):
    import ml_dtypes

    x = np.asarray(x, np.float32)
    ln_g = np.asarray(ln_g, np.float32)
    ln_b = np.asarray(ln_b, np.float32)
    w_qkv = np.asarray(w_qkv, np.float32)
    w_out = np.asarray(w_out, np.float32)
    b_out = np.asarray(b_out, np.float32)

    cos2, sin2 = _rope_tables()
    pmatT = _perm_matrix()
    ident = np.eye(128, dtype=np.float32)
    has_lnb = bool(np.any(ln_b != 0.0))

    def fold(mat):  # [1024, 512] -> [128, 8, 512]
        return np.ascontiguousarray(mat.reshape(KC, 128, 512).transpose(1, 0, 2))

    in_maps = []
    for c in range(N_CORES):
        b, g = c // 2, c % 2
        qs = slice(g * 512, (g + 1) * 512)
        ks = slice(DIM + g * 512, DIM + (g + 1) * 512)
        vs = slice(2 * DIM + g * 512, 2 * DIM + (g + 1) * 512)
        wq = fold(ln_g[:, None] * w_qkv[:, qs]) / 8.0
        wk = fold(ln_g[:, None] * w_qkv[:, ks])
        wv = fold(ln_g[:, None] * w_qkv[:, vs])
        bq = (ln_b @ w_qkv[:, qs]) / 8.0
        bk = ln_b @ w_qkv[:, ks]
        bv = ln_b @ w_qkv[:, vs]
        wo = np.ascontiguousarray(
            w_out[g * 512:(g + 1) * 512, :].reshape(4, 128, DIM).transpose(1, 0, 2)
        ).astype(ml_dtypes.bfloat16)
        bias_full = (b_out if g == 0 else np.zeros_like(b_out)).reshape(1, DIM)
        in_maps.append({
            "x": np.ascontiguousarray(x[b]),
            "wq": wq.astype(ml_dtypes.bfloat16),
            "wk": wk.astype(ml_dtypes.bfloat16),
            "wv": wv.astype(ml_dtypes.bfloat16),
            "wo": wo,
            "bq": np.ascontiguousarray(bq.reshape(4, 128).T).astype(np.float32),
            "bk": np.ascontiguousarray(bk.reshape(4, 128).T).astype(np.float32),
            "bv": np.ascontiguousarray(bv.reshape(NH_C, 64).T).astype(np.float32),
            "cos2": cos2.astype(np.float32),
            "sin2": sin2.astype(np.float32),
            "pmatT": pmatT,
            "ident": ident,
            "bias_bc": np.ascontiguousarray(np.broadcast_to(bias_full, (128, DIM))).astype(np.float32),
        })
    return in_maps, has_lnb


def _get_program(has_lnb, reps=1):
    key = ("prog", has_lnb, reps)
    if key not in _cache:
        _cache[key] = _build_program(has_lnb, reps)
    return _cache[key]


def kernel(**inputs):
    from concourse.bass_utils import run_bass_kernel_spmd

    in_maps, has_lnb = _prep_inputs(**inputs)
    nc = _get_program(has_lnb)
    res = run_bass_kernel_spmd(nc, in_maps, list(range(N_CORES))).results
    out = np.empty((B, SEQ, DIM), np.float32)
    for b in range(B):
        out[b] = res[2 * b]["out_p"] + res[2 * b + 1]["out_p"]
    return out
